# revision 2
# baseline (speedup 1.0000x reference)
"""Trainium2 Bass kernel for DiagonalLinear.

The reference masks W to its diagonal (zeroing entries with |w| <= 1e-4)
and computes x @ masked_W.T, which is exactly an elementwise scale of
x's columns by the thresholded diagonal of W.

Distribution (8 NeuronCores): data-parallel — x is sharded along the
token axis (1024 tokens per core); per the sharding hint, only the
(thresholded) diagonal of W — 4096 floats, the sole part of W the op
reads — is replicated to every core. No inter-core communication.

The op is purely memory-bound. bf16 streaming (x and out quantized on
the host; three roundings stay under 1.2%, inside the 2e-2 tolerance)
puts per-core traffic at 8 MiB in + 8 MiB out. The combined load+store
stream saturates the 16 SBUF AXI ports (~425 GB/s measured), so the
floor is ~40 us of streaming plus ~8.5 us of fixed framework
preamble/postamble. This version restructures the baseline's schedule
to keep the pipe at that rate the whole time:

1. The d-row load rides the SCALAR (qAct) ring at t=0, in parallel
   with the x-tile loads on the sync (qSP) ring, and the diagonal
   broadcast (8 exact K=1 matmuls ones^T @ d_row -> PSUM banks) is
   copied to SBUF by the DVE per 512-column chunk as each matmul
   lands. First multiply fires at ~6 us (vs ~13.4 us when the d row
   queued behind x-tile packets), so the store stream is flowing by
   ~7 us and load/store packets interleave for the rest of the kernel.

2. Tile order [128,128,128,120,120,128,128,128,16]: the 120-row tiles
   (15-engine, port-crossed descriptor layout — full rate only when
   load and store packets interleave) run mid-stream where stores are
   already active, instead of in the load-only window where they
   measured ~215 GB/s. The [128] tiles (port-aligned, full rate even
   single-queue) bracket the stream. Engine 15 (the ~18%-slower SDMA
   engine) gets 49/65 of the per-engine line count, matching its speed.

3. The tiny [16]-row tile goes LAST: the final load->mul->store
   dependency chain after the last load byte is ~2.8 us (sem receipt +
   a 0.14 us multiply + issue), fully hidden under the remaining
   store-backlog drain, so the kernel ends at bytes-end + one HBM
   write receipt.

4. Multiplies and stores run per column-half (512 KB units) so the
   store FIFO is fed at fine granularity; the last three store units
   ride the sync ring once its loads have drained.

Per-core device program — raw Bass (no Tile scheduler) with hand-placed
semaphores; the kernel ends on store-completion waits, not an
all-engine barrier.

Engine plan (single Block, all engines concurrent):
  sync   : 9 x-tile loads on the qSP ring, a write-path warm-up, then
           the last 3 store units
  scalar : d-row load at t=0 on the qAct ring, a write-path warm-up,
           then store units 0..14 (each gated on its multiply)
  tensor : 8 K=1 matmuls ones[1,128]^T @ d_row[1,512] -> PSUM banks
  vector : 8 PSUM->SBUF bf16 copies of the replicated diagonal (one
           per matmul, chunk-pipelined), then the 18 tile multiplies
"""

import numpy as np

TOKENS = 8192
N = 4096
N_CORES = 8
T_SHARD = TOKENS // N_CORES  # 1024
TILE_P = [128, 128, 128, 120, 120, 128, 128, 128, 16]
P0 = max(TILE_P)
MM_N = 512                   # PSUM bank width (fp32)
THRESHOLD = 1e-4
N_SYNC_STORES = 3            # tail store units issued on the sync ring

_CACHED_NC = None


def _build_nc(tile_p=None, n_sync_stores=N_SYNC_STORES):
    from contextlib import ExitStack

    from concourse import bass, mybir

    bf16 = mybir.dt.bfloat16
    f32 = mybir.dt.float32
    tile_p = list(TILE_P) if tile_p is None else list(tile_p)
    nc = bass.Bass()
    x_in = nc.declare_dram_parameter("x", [T_SHARD, N], bf16, isOutput=False)
    d_in = nc.declare_dram_parameter("d", [N], bf16, isOutput=False)
    out = nc.declare_dram_parameter("out", [T_SHARD, N], bf16, isOutput=True)
    warm = nc.dram_tensor("warm", [2, N], bf16)  # write-path warm-up target

    x_ap = x_in[:]
    o_ap = out[:]
    offs = np.cumsum([0] + tile_p)
    x_v = [x_ap[offs[i] : offs[i + 1]] for i in range(len(tile_p))]
    o_v = [o_ap[offs[i] : offs[i + 1]] for i in range(len(tile_p))]

    n_tiles = len(tile_p)
    H = N // 2
    # (tile, col_slice) units in mul/store order: two column-halves per tile
    units = [(t, slice(h * H, (h + 1) * H)) for t in range(n_tiles) for h in range(2)]
    n_mul = len(units)                    # 18
    n_scalar_units = n_mul - n_sync_stores

    with ExitStack() as ctx:
        s_ld = [
            ctx.enter_context(nc.semaphore(f"s_ld{i}")) for i in range(n_tiles)
        ]
        s_row = ctx.enter_context(nc.semaphore("s_row"))
        s_ones = ctx.enter_context(nc.semaphore("s_ones"))
        s_mm = ctx.enter_context(nc.semaphore("s_mm"))
        s_cp = ctx.enter_context(nc.semaphore("s_cp"))
        s_mul = ctx.enter_context(nc.semaphore("s_mul"))
        s_st = ctx.enter_context(nc.semaphore("s_st"))
        s_st2 = ctx.enter_context(nc.semaphore("s_st2"))
        s_warm = ctx.enter_context(nc.semaphore("s_warm"))

        row = ctx.enter_context(nc.sbuf_tensor("row", [1, N], bf16))
        ones = ctx.enter_context(nc.sbuf_tensor("ones", [1, P0], bf16))
        db = ctx.enter_context(nc.sbuf_tensor("db", [P0, N], bf16))
        xts = [
            ctx.enter_context(nc.sbuf_tensor(f"xt{i}", [p, N], bf16))
            for i, p in enumerate(tile_p)
        ]
        acc = ctx.enter_context(nc.psum_tensor("acc", [P0, N], f32))

        with nc.Block() as block:

            @block.sync
            def _(sync):
                for i in range(n_tiles):
                    sync.dma_start(out=xts[i][:], in_=x_v[i]).then_inc(s_ld[i], 16)
                # s_row long satisfied by now (d rode the idle qAct ring)
                sync.wait_ge(s_row, 16)
                sync.dma_start(out=warm[0, None, :], in_=row[:]).then_inc(
                    s_warm, 16
                )
                # tail stores ride the sync ring: it is idle once the
                # loads drain, so the store backlog drains on both rings
                for k in range(n_scalar_units, n_mul):
                    t, cs = units[k]
                    sync.wait_ge(s_mul, k + 1)
                    sync.dma_start(out=o_v[t][:, cs], in_=xts[t][:, cs]).then_inc(
                        s_st2, 16
                    )
                sync.wait_ge(s_st2, 16 * n_sync_stores)
                sync.wait_ge(s_warm, 32)

            @block.scalar
            def _(scalar):
                # d-row load heads the (otherwise idle) qAct ring: its 16
                # descriptors land in the first packet round (~1 us)
                # while the x tiles stream on qSP
                scalar.dma_start(out=row[:], in_=d_in[None, :]).then_inc(s_row, 16)
                scalar.wait_ge(s_row, 16)
                scalar.dma_start(out=warm[1, None, :], in_=row[:]).then_inc(
                    s_warm, 16
                )
                for k in range(n_scalar_units):
                    t, cs = units[k]
                    scalar.wait_ge(s_mul, k + 1)
                    scalar.dma_start(
                        out=o_v[t][:, cs], in_=xts[t][:, cs]
                    ).then_inc(s_st, 16)
                scalar.wait_ge(s_st, 16 * n_scalar_units)
                scalar.wait_ge(s_warm, 32)

            @block.tensor
            def _(tensor):
                tensor.wait_ge(s_ones, 1)
                tensor.wait_ge(s_row, 16)
                for j in range(N // MM_N):
                    tensor.matmul(
                        acc[:, j * MM_N : (j + 1) * MM_N],
                        ones[:],
                        row[:, j * MM_N : (j + 1) * MM_N],
                        start=True,
                        stop=True,
                    ).then_inc(s_mm, 1)

            @block.vector
            def _(vector):
                vector.memset(ones[:], 1.0).then_inc(s_ones, 1)
                # PSUM -> SBUF bf16 broadcast copies, chunk-pipelined
                # behind the matmuls (exact: f32 holds the bf16 values)
                for j in range(N // MM_N):
                    vector.wait_ge(s_mm, j + 1)
                    vector.tensor_copy(
                        out=db[:, j * MM_N : (j + 1) * MM_N],
                        in_=acc[:, j * MM_N : (j + 1) * MM_N],
                    ).then_inc(s_cp, 1)
                seen = set()
                for k, (t, cs) in enumerate(units):
                    p = tile_p[t]
                    if t not in seen:
                        seen.add(t)
                        vector.wait_ge(s_ld[t], 16)
                    vector.tensor_mul(
                        out=xts[t][:, cs], in0=xts[t][:, cs], in1=db[:p, cs]
                    ).then_inc(s_mul, 1)

    nc.finalize()
    return nc


def _get_nc():
    global _CACHED_NC
    if _CACHED_NC is None:
        _CACHED_NC = _build_nc()
    return _CACHED_NC


def _shard_inputs(x, W):
    import ml_dtypes

    bf16 = ml_dtypes.bfloat16
    x = np.ascontiguousarray(np.asarray(x, dtype=np.float32)).astype(bf16)
    W = np.asarray(W, dtype=np.float32)
    d = np.ascontiguousarray(np.diagonal(W))
    d = np.where(np.abs(d) > THRESHOLD, d, np.float32(0.0)).astype(bf16)
    assert x.shape == (TOKENS, N) and d.shape == (N,)
    return [
        {"x": x[c * T_SHARD : (c + 1) * T_SHARD], "d": d} for c in range(N_CORES)
    ]


def _run(x, W, **spmd_kwargs):
    from concourse.bass_utils import run_bass_kernel_spmd

    nc = _get_nc()
    in_maps = _shard_inputs(x, W)
    res = run_bass_kernel_spmd(nc, in_maps, list(range(N_CORES)), **spmd_kwargs)
    out = np.concatenate(
        [res.results[c]["out"] for c in range(N_CORES)], axis=0
    ).astype(np.float32)
    return out, res


def kernel(x, W):
    out, _ = _run(x, W)
    return out


# revision 8
# speedup vs baseline: 1.0766x; 1.0766x over previous
"""Trainium2 Bass kernel for DiagonalLinear.

The reference masks W to its diagonal (zeroing entries with |w| <= 1e-4)
and computes x @ masked_W.T, which is exactly an elementwise scale of
x's columns by the thresholded diagonal of W.

Distribution (8 NeuronCores): data-parallel — x is sharded along the
token axis (1024 tokens per core); per the sharding hint, only the
(thresholded) diagonal of W — 4096 floats, the sole part of W the op
reads — is replicated to every core. No inter-core communication.

The op is purely memory-bound. bf16 streaming (x and out quantized on
the host; three roundings stay under 1.2%, inside the 2e-2 tolerance)
puts per-core traffic at 8 MiB in + 8 MiB out. The combined load+store
stream saturates the 16 SBUF AXI ports (~425 GB/s measured), so the
floor is ~40 us of streaming plus ~8.5 us of fixed framework
preamble/postamble. This version restructures the baseline's schedule
to keep the pipe at that rate the whole time:

1. A ring's FIRST DMA pays a ~4.5 us (qAct) / ~1.5 us (qSP) cold
   start before bytes move (measured). The scalar engine therefore
   issues a no-wait dummy write (uninitialized scratch -> DRAM) at
   t=0 so the qAct ring is warm before the first store needs it; the
   d-row load stays at the HEAD of the sync ring where the cold cost
   overlaps the x-tile stream behind it.

2. A tiny [16]-row tile loads FIRST: its 128 KB land ~4.6 us in, and
   the DVE interleaves the diagonal-broadcast PSUM->SBUF copies with
   the first multiply (chunk-gated on the 8 K=1 matmuls
   ones^T @ d_row -> PSUM), so the first store issues at ~6 us
   instead of ~15.8 us — load and store packets interleave for
   essentially the whole stream.

3. Tile order [16,128,128,120,120,128,128,128,128]: the 120-row tiles
   (15-engine, port-crossed descriptor layout — full rate only when
   load and store packets interleave) run mid-stream where stores are
   already active, instead of in the load-only window where they
   measured ~215 GB/s. Engine 15 (the ~18%-slower SDMA engine) gets
   49/65 of the per-engine line count, matching its speed.

4. Multiplies and stores run per column-half (512 KB units) so the
   store FIFO is fed at fine granularity; the last three store units
   ride the sync ring once its loads have drained, and the final
   load->mul->store chain hides under the store-backlog drain.

Per-core device program — raw Bass (no Tile scheduler) with hand-placed
semaphores; the kernel ends on store-completion waits, not an
all-engine barrier.

Engine plan (single Block, all engines concurrent):
  sync   : d-row load, 9 x-tile loads on the qSP ring, a warm-up
           write, then the last 3 store units
  scalar : no-wait qAct warm-up write at t=0, then store units 0..14
           (each gated on its multiply)
  tensor : 8 K=1 matmuls ones[1,128]^T @ d_row[1,512] -> PSUM banks
  vector : PSUM->SBUF bf16 copies of the replicated diagonal (one per
           matmul, chunk-pipelined, interleaved with the first
           multiply), then the remaining tile multiplies
"""

import numpy as np

TOKENS = 8192
N = 4096
N_CORES = 8
T_SHARD = TOKENS // N_CORES  # 1024
TILE_P = [16, 128, 128, 120, 120, 128, 128, 128, 128]
P0 = max(TILE_P)
MM_N = 512                   # PSUM bank width (fp32)
THRESHOLD = 1e-4
N_SYNC_STORES = 3            # tail store units issued on the sync ring

_CACHED_NC = None


def _build_nc(tile_p=None, n_sync_stores=N_SYNC_STORES):
    from contextlib import ExitStack

    from concourse import bass, mybir

    bf16 = mybir.dt.bfloat16
    f32 = mybir.dt.float32
    tile_p = list(TILE_P) if tile_p is None else list(tile_p)
    nc = bass.Bass()
    x_in = nc.declare_dram_parameter("x", [T_SHARD, N], bf16, isOutput=False)
    d_in = nc.declare_dram_parameter("d", [N], bf16, isOutput=False)
    out = nc.declare_dram_parameter("out", [T_SHARD, N], bf16, isOutput=True)
    warm = nc.dram_tensor("warm", [2, N], bf16)  # write-path warm-up target

    x_ap = x_in[:]
    o_ap = out[:]
    offs = np.cumsum([0] + tile_p)
    x_v = [x_ap[offs[i] : offs[i + 1]] for i in range(len(tile_p))]
    o_v = [o_ap[offs[i] : offs[i + 1]] for i in range(len(tile_p))]

    n_tiles = len(tile_p)
    H = N // 2
    # (tile, col_slice) units in mul/store order: two column-halves per tile
    units = [(t, slice(h * H, (h + 1) * H)) for t in range(n_tiles) for h in range(2)]
    n_mul = len(units)                    # 18
    n_scalar_units = n_mul - n_sync_stores

    with ExitStack() as ctx:
        s_ld = [
            ctx.enter_context(nc.semaphore(f"s_ld{i}")) for i in range(n_tiles)
        ]
        s_row = ctx.enter_context(nc.semaphore("s_row"))
        s_ones = ctx.enter_context(nc.semaphore("s_ones"))
        s_mm = ctx.enter_context(nc.semaphore("s_mm"))
        s_cp = ctx.enter_context(nc.semaphore("s_cp"))
        s_mul = ctx.enter_context(nc.semaphore("s_mul"))
        s_st = ctx.enter_context(nc.semaphore("s_st"))
        s_st2 = ctx.enter_context(nc.semaphore("s_st2"))
        s_warm = ctx.enter_context(nc.semaphore("s_warm"))

        row = ctx.enter_context(nc.sbuf_tensor("row", [1, N], bf16))
        ones = ctx.enter_context(nc.sbuf_tensor("ones", [1, P0], bf16))
        db = ctx.enter_context(nc.sbuf_tensor("db", [P0, N], bf16))
        # dedicated never-written scratch: the t=0 qAct warm-up reads it
        # (contents irrelevant; target is DRAM scratch)
        wsrc = ctx.enter_context(nc.sbuf_tensor("wsrc", [1, N], bf16))
        xts = [
            ctx.enter_context(nc.sbuf_tensor(f"xt{i}", [p, N], bf16))
            for i, p in enumerate(tile_p)
        ]
        acc = ctx.enter_context(nc.psum_tensor("acc", [P0, N], f32))

        with nc.Block() as block:

            @block.sync
            def _(sync):
                # d-row load heads the sync FIFO: its 16 descriptors
                # complete in the first packet round instead of queueing
                # behind x-tile packets
                sync.dma_start(out=row[:], in_=d_in[None, :]).then_inc(s_row, 16)
                for i in range(n_tiles):
                    sync.dma_start(out=xts[i][:], in_=x_v[i]).then_inc(s_ld[i], 16)
                # s_row long satisfied by now
                sync.wait_ge(s_row, 16)
                sync.dma_start(out=warm[0, None, :], in_=row[:]).then_inc(
                    s_warm, 16
                )
                # tail stores ride the sync ring: it is idle once the
                # loads drain, so the store backlog drains on both rings
                for k in range(n_scalar_units, n_mul):
                    t, cs = units[k]
                    sync.wait_ge(s_mul, k + 1)
                    sync.dma_start(out=o_v[t][:, cs], in_=xts[t][:, cs]).then_inc(
                        s_st2, 16
                    )
                sync.wait_ge(s_st2, 16 * n_sync_stores)
                sync.wait_ge(s_warm, 32)

            @block.scalar
            def _(scalar):
                # no-wait warm-up: first DMA on the qAct ring pays a
                # ~4.5 us cold start, so burn it at t=0 on a dummy write
                # (uninitialized scratch -> DRAM scratch) instead of on
                # the first output store
                scalar.dma_start(out=warm[1, None, :], in_=wsrc[:]).then_inc(
                    s_warm, 16
                )
                for k in range(n_scalar_units):
                    t, cs = units[k]
                    scalar.wait_ge(s_mul, k + 1)
                    scalar.dma_start(
                        out=o_v[t][:, cs], in_=xts[t][:, cs]
                    ).then_inc(s_st, 16)
                scalar.wait_ge(s_st, 16 * n_scalar_units)
                scalar.wait_ge(s_warm, 32)

            @block.tensor
            def _(tensor):
                tensor.wait_ge(s_ones, 1)
                tensor.wait_ge(s_row, 16)
                for j in range(N // MM_N):
                    tensor.matmul(
                        acc[:, j * MM_N : (j + 1) * MM_N],
                        ones[:],
                        row[:, j * MM_N : (j + 1) * MM_N],
                        start=True,
                        stop=True,
                    ).then_inc(s_mm, 1)

            @block.vector
            def _(vector):
                vector.memset(ones[:], 1.0).then_inc(s_ones, 1)

                # PSUM -> SBUF bf16 broadcast copies, chunk-pipelined
                # behind the matmuls (exact: f32 holds the bf16 values).
                # The first multiply (the tiny 16-row tile's first half)
                # is interleaved after the 4 chunks it needs so the store
                # stream starts while the last matmuls are still running.
                def copy_chunks(lo, hi):
                    for j in range(lo, hi):
                        vector.wait_ge(s_mm, j + 1)
                        vector.tensor_copy(
                            out=db[:, j * MM_N : (j + 1) * MM_N],
                            in_=acc[:, j * MM_N : (j + 1) * MM_N],
                        ).then_inc(s_cp, 1)

                seen = set()

                def mul_unit(k):
                    t, cs = units[k]
                    p = tile_p[t]
                    if t not in seen:
                        seen.add(t)
                        vector.wait_ge(s_ld[t], 16)
                    vector.tensor_mul(
                        out=xts[t][:, cs], in0=xts[t][:, cs], in1=db[:p, cs]
                    ).then_inc(s_mul, 1)

                copy_chunks(0, 4)
                mul_unit(0)
                copy_chunks(4, 8)
                for k in range(1, n_mul):
                    mul_unit(k)

    nc.finalize()
    return nc


def _get_nc():
    global _CACHED_NC
    if _CACHED_NC is None:
        _CACHED_NC = _build_nc()
    return _CACHED_NC


def _shard_inputs(x, W):
    import ml_dtypes

    bf16 = ml_dtypes.bfloat16
    x = np.ascontiguousarray(np.asarray(x, dtype=np.float32)).astype(bf16)
    W = np.asarray(W, dtype=np.float32)
    d = np.ascontiguousarray(np.diagonal(W))
    d = np.where(np.abs(d) > THRESHOLD, d, np.float32(0.0)).astype(bf16)
    assert x.shape == (TOKENS, N) and d.shape == (N,)
    return [
        {"x": x[c * T_SHARD : (c + 1) * T_SHARD], "d": d} for c in range(N_CORES)
    ]


def _run(x, W, **spmd_kwargs):
    from concourse.bass_utils import run_bass_kernel_spmd

    nc = _get_nc()
    in_maps = _shard_inputs(x, W)
    res = run_bass_kernel_spmd(nc, in_maps, list(range(N_CORES)), **spmd_kwargs)
    out = np.concatenate(
        [res.results[c]["out"] for c in range(N_CORES)], axis=0
    ).astype(np.float32)
    return out, res


def kernel(x, W):
    out, _ = _run(x, W)
    return out
